# revision 31
# baseline (speedup 1.0000x reference)
"""Trainium2 Bass kernel for 16-head causal MHA (B=2, T=2048, C=1024, H=16, D=64).

Sharding: 8 cores = 2 batch groups x 4 head groups (4 heads each).

v2: fp8e4 DoubleRow matmuls for the projections (c-tile pairs, contraction
256/instr at 0.5 cyc/row) and for the AV accumulation (ts-tile pairs).
Scores and out-projection stay bf16 in plain 128x128 mode, so the PE never
switches tiling modes.

Per-core pipeline:
  Q^T,K^T = fp8-DR projections kept transposed [dims, tokens], bf16 out
            (Wq pre-scaled by softmax-scale*1024, Wk by 32 on host)
  V       = fp8-DR projection, natural [tokens, dims], stored fp8 (Wv*32)
            in ts-tile-PAIR layout vaug[pair][:, slab, head, 0:64] plus a
            ones column per head (denominator trick)
  S^T     = bf16 K Q^T per (ts-tile, tq-chunk), causal-masked on the
            diagonal block, exp'd with scale 2^-15 (undoes S_q*S_k) into
            fp8 pt pairs
  O^T_aug = fp8-DR V_aug^T P^T over ts-tile pairs; row 64 is the softmax
            denominator; normalized via partition_broadcast + DVE
  Y_part  = O^T.T @ (Wo_slice/32)^T bf16, interleaved per chunk
Host sums the 4 head-group partials per batch and adds bo.
"""

import sys

sys.path.insert(0, "/opt/trn_rl_repo")

import numpy as np

import concourse.bass as bass
from concourse import bacc
import concourse.mybir as mybir
from concourse.tile import TileContext
from concourse.bass_utils import run_bass_kernel_spmd

F32 = mybir.dt.float32
BF16 = mybir.dt.bfloat16
F8 = mybir.dt.float8e4
DR = mybir.MatmulPerfMode.DoubleRow
EXP = mybir.ActivationFunctionType.Exp

B, T, C, H, D = 2, 2048, 1024, 16, 64
NHPC = 4          # heads per core
DH = NHPC * D     # 256 head dims per core
P = 128           # partitions
CH = 512          # token chunk (matmul moving dim)
NCHUNK = T // CH  # 4
NTT = T // P      # 16 token tiles
NCP = C // (2 * P)  # 4 contraction PAIRS over C (DoubleRow slabs)
# host-side scale factors keeping fp8 operands in e4m3 normal range
S_Q = 1024.0      # on Wq (includes softmax 1/32): exp scale undoes S_Q*S_K
S_K = 32.0
S_V = 32.0        # on Wv: undone by Wo/32 on host
EXP_SCALE = 1.0 / (S_Q * S_K)


def build_nc(stages=3, no_mask=False, no_norm=False):
    nc = bacc.Bacc()
    # x^T in c-tile-pair layout: [cpair, c-within(128 part), slab, tokens]
    x8_d = nc.declare_dram_parameter("X8", [NCP, P, 2, T], F8, isOutput=False)
    # Wqkv fp8 in c-tile-pair layout: cols 0:256 Q, 256:512 K, 512:768 V
    w8_d = nc.declare_dram_parameter("W8", [NCP, P, 2, 3 * DH], F8,
                                     isOutput=False)
    wot_d = nc.declare_dram_parameter("WoT", [DH, C], BF16, isOutput=False)
    # bf16 x^T (first 512 tokens) and Wv for the accurate early-V path
    xtb_d = nc.declare_dram_parameter("XTB", [C, CH], BF16, isOutput=False)
    wvb_d = nc.declare_dram_parameter("WVB", [C, DH], BF16, isOutput=False)
    y_d = nc.declare_dram_parameter("Y", [T, C], BF16, isOutput=True)

    wot = wot_d[:, :]
    y = y_d[:, :]

    with TileContext(nc) as tc:
        with (
            tc.tile_pool(name="const", bufs=1) as const,
            tc.tile_pool(name="persist", bufs=1) as persist,
        ):
            # ---- constants ----
            ones_f32 = const.tile([P, 1], F32)
            nc.gpsimd.memset(ones_f32[:], 1.0)
            # warm the GPSIMD library that partition_broadcast lives in
            wsrc = const.tile([1, 8], F32, name="warmsrc")
            wdst = const.tile([D, 8], F32, name="warmdst")
            nc.gpsimd.memset(wsrc[:], 1.0)
            nc.gpsimd.partition_broadcast(wdst[:], wsrc[:])
            nc.vector.tensor_copy(wsrc[:], wdst[0:1, :])
            # triangular mask for the diagonal 128x128 block (both heads):
            # mask[r, (hh, j)] = 1 if r <= j else 0, applied to fp8 probs
            mask01 = const.tile([P, 2, P], F8, name="mask01")
            mask01b = const.tile([P, 2, P], BF16, name="mask01b")
            for mt in (mask01, mask01b):
                nc.gpsimd.memset(mt[:], 1.0)
                nc.gpsimd.affine_select(
                    out=mt[:],
                    in_=mt[:],
                    compare_op=mybir.AluOpType.is_ge,
                    fill=0.0,
                    base=0,
                    pattern=[[0, 2], [1, P]],
                    channel_multiplier=-1,
                )

            # ---- persistent tensors ----
            w8_t = [persist.tile([P, 2, 3 * DH], F8, name=f"w8_{c}")
                    for c in range(NCP)]
            wot_t = [persist.tile([P, C], BF16, name=f"wot{k}") for k in range(2)]
            # Q^T/K^T [dims, tokens] bf16; pair p holds heads (2p, 2p+1)
            qt_t = [persist.tile([P, T], BF16, name=f"qt{p}") for p in range(2)]
            kt_t = [persist.tile([P, T], BF16, name=f"kt{p}") for p in range(2)]
            # V fp8 in ts-tile-pair layout + ones column per head:
            # vaug[k][:, slab, h, 0:64] = V' rows for ts-tile 2k+slab
            vaug_t = [persist.tile([P, 2, NHPC, 2 * D], F8, name=f"vaug{k}")
                      for k in range(NTT // 2)]
            # bf16 V for ts-tiles 0-3 (queries < 512 need accurate V)
            vaugb_t = [persist.tile([P, NHPC, D + 1], BF16, name=f"vaugb{t}")
                       for t in range(4)]
            # ones column at slot 64: the denominator lands on PSUM
            # partition 64 (32-aligned), where reciprocal_approx_fast reads
            # it directly -- no staging copy
            with nc.allow_low_precision("fp8 attention pipeline"):
                for k in range(NTT // 2):
                    nc.gpsimd.memset(vaug_t[k][:, :, :, D : D + 1], 1.0)
                for t in range(4):
                    nc.gpsimd.memset(vaugb_t[t][:, :, D : D + 1], 1.0)
            # normalized O^T [dims, tokens] bf16
            ot_t = [persist.tile([P, T], BF16, name=f"ot{p}") for p in range(2)]

            def emit_stage1():
                with (
                    tc.tile_pool(name="xt", bufs=12) as xt_pool,
                    tc.tile_pool(name="xtb", bufs=8) as xtb_pool,
                    tc.tile_pool(name="psproj", bufs=4, space="PSUM") as ps_proj,
                ):
                    xtb_ts = []
                    wvb_ts = []
                    for n in range(NCHUNK):
                        csl = slice(n * CH, (n + 1) * CH)
                        xts = []
                        for c in range(NCP):
                            if n == 0:
                                nc.sync.dma_start(
                                    w8_t[c][:], w8_d[c, :, :, :])
                            xtile = xt_pool.tile([P, 2, CH], F8, tag="xt",
                                                 name=f"xt{n}_{c}")
                            nc.scalar.dma_start(xtile[:],
                                                x8_d[c, :, :, csl])
                            xts.append(xtile)
                        if n == 0:
                            for k in range(2):
                                nc.sync.dma_start(wot_t[k][:],
                                                  wot[k * P : (k + 1) * P, :])
                            for c8 in range(8):
                                xb = xtb_pool.tile([P, CH], BF16, tag="xb",
                                                   name=f"xtb{c8}")
                                nc.gpsimd.dma_start(
                                    xb[:], xtb_d[c8 * P : (c8 + 1) * P, :])
                                xtb_ts.append(xb)
                                wb = xtb_pool.tile([P, DH], BF16, tag="wb",
                                                   name=f"wvb{c8}")
                                nc.gpsimd.dma_start(
                                    wb[:], wvb_d[c8 * P : (c8 + 1) * P, :])
                                wvb_ts.append(wb)
                        # Q^T/K^T: W stationary, x^T moving -> [dims, tokens]
                        for m in range(4):
                            ps = ps_proj.tile([P, CH], F32, tag="ps",
                                              name=f"ps{n}_{m}")
                            for c in range(NCP):
                                nc.tensor.matmul(
                                    ps[:],
                                    lhsT=w8_t[c][:, :, m * P : (m + 1) * P],
                                    rhs=xts[c][:],
                                    start=(c == 0),
                                    stop=(c == NCP - 1),
                                    perf_mode=DR,
                                )
                            with nc.allow_low_precision("bf16 Q/K"):
                                if m < 2:
                                    nc.vector.tensor_copy(qt_t[m][:, csl], ps[:])
                                else:
                                    nc.scalar.copy(kt_t[m - 2][:, csl], ps[:])
                        # V natural: x^T tile stationary, Wv moving.
                        # Chunk 0 (ts-tiles 0-3) uses bf16 inputs: those V
                        # rows weight early queries heavily and must be
                        # accurate.  It feeds both the bf16 vaugb tiles and
                        # the fp8 pair tiles.
                        for j in range(4):
                            t_glob = 4 * n + j
                            vp = ps_proj.tile([P, DH], F32, tag="vp",
                                              name=f"vp{n}_{j}")
                            if n == 0:
                                for c8 in range(8):
                                    nc.tensor.matmul(
                                        vp[:],
                                        lhsT=xtb_ts[c8][:, j * P : (j + 1) * P],
                                        rhs=wvb_ts[c8][:],
                                        start=(c8 == 0),
                                        stop=(c8 == 7),
                                    )
                            else:
                                for c in range(NCP):
                                    nc.tensor.matmul(
                                        vp[:],
                                        lhsT=xts[c][:, :, j * P : (j + 1) * P],
                                        rhs=w8_t[c][:, :, 2 * DH : 3 * DH],
                                        start=(c == 0),
                                        stop=(c == NCP - 1),
                                        perf_mode=DR,
                                    )
                            # strided copies: heads land at 65/128-col slots
                            va = vaug_t[t_glob // 2]
                            with nc.allow_low_precision("fp8 V"):
                                nc.vector.tensor_copy(
                                    va[:, t_glob % 2, :, 0:D],
                                    vp[:].rearrange("p (h d) -> p h d",
                                                    h=NHPC))
                                if n == 0:
                                    nc.scalar.copy(
                                        vaugb_t[j][:, :, 0:D],
                                        vp[:].rearrange("p (h d) -> p h d",
                                                        h=NHPC))

            def emit_stage23():
                with (
                    tc.tile_pool(name="pt", bufs=4) as pt_pool,
                    tc.tile_pool(name="small", bufs=4) as small_pool,
                    tc.tile_pool(name="ysb", bufs=3) as y_pool,
                    tc.tile_pool(name="psst", bufs=2, space="PSUM") as ps_st,
                    tc.tile_pool(name="psot", bufs=3, space="PSUM") as ps_ot,
                    tc.tile_pool(name="psy", bufs=1, space="PSUM") as ps_y,
                ):
                    pending_out = []

                    def emit_outproj_sub(sub, on_scalar=False):
                        tt, nn = sub
                        tsl = slice(tt * P, (tt + 1) * P)
                        nsl = slice(nn * CH, (nn + 1) * CH)
                        yp = ps_y.tile([P, CH], F32, tag="y",
                                       name=f"y{tt}_{nn}")
                        for k in range(2):
                            nc.tensor.matmul(
                                yp[:],
                                lhsT=ot_t[k][:, tsl],
                                rhs=wot_t[k][:, nsl],
                                start=(k == 0),
                                stop=(k == 1),
                            )
                        ysb = y_pool.tile([P, CH], BF16, tag="ysb",
                                          name=f"ysb{tt}_{nn}")
                        with nc.allow_low_precision("bf16 Y"):
                            if on_scalar:
                                nc.scalar.copy(ysb[:], yp[:])
                            else:
                                nc.vector.tensor_copy(ysb[:], yp[:])
                        nc.sync.dma_start(y[tsl, nsl], ysb[:])

                    def emit_st(cq, p, t):
                        st = ps_st.tile([P, 2, CH], F32, tag="st",
                                        name=f"st{cq}_{p}_{t}")
                        tsl = slice(t * P, (t + 1) * P)
                        js = max(0, (t - 4 * cq) * P)
                        qs = slice(cq * CH + js, (cq + 1) * CH)
                        for hh in range(2):
                            nc.tensor.matmul(
                                st[:, hh, js:],
                                lhsT=kt_t[p][hh * D : (hh + 1) * D, tsl],
                                rhs=qt_t[p][hh * D : (hh + 1) * D, qs],
                                start=True,
                                stop=True,
                            )
                        return st, js

                    def norm_steps(cq, p, ots, on_scalar=False):
                        """Normalize chain for (cq, p) as separate steps so
                        they interleave with the pipeline on the DVE queue."""
                        qsl = slice(cq * CH, (cq + 1) * CH)
                        if no_norm:
                            def all_copies():
                                with nc.allow_low_precision("timing variant"):
                                    for hh in range(2):
                                        nc.vector.tensor_copy(
                                            ot_t[p][hh * D : (hh + 1) * D, qsl],
                                            ots[hh][0:D, :])
                            return [all_copies]
                        cp = nc.scalar.copy if on_scalar else nc.vector.tensor_copy
                        state = {}

                        def phase1(hh):
                            def fn():
                                ot = ots[hh]
                                # reciprocal reads the denominator row
                                # directly from PSUM partition 0; the otu
                                # bounce frees the PSUM slot quickly
                                dstg = small_pool.tile([1, CH], F32,
                                                       tag="ds",
                                                       name=f"ds{cq}_{p}_{hh}")
                                cp(dstg[:], ot[D : D + 1, :])
                                recip = small_pool.tile([1, CH], F32, tag="rc",
                                                        name=f"rc{cq}_{p}_{hh}")
                                nc.vector.reciprocal_approx_fast(
                                    recip[:], dstg[:])
                                otu = small_pool.tile([D, CH], F32,
                                                      tag="otu",
                                                      name=f"otu{cq}_{p}_{hh}")
                                cp(otu[:], ot[0:D, :])
                                state[hh] = (recip, otu)
                            return fn

                        def recips():
                            pass

                        def phase2(hh):
                            def fn():
                                recip, otu = state[hh]
                                den = small_pool.tile([D, CH], F32, tag="den",
                                                      name=f"dn{cq}_{p}_{hh}")
                                nc.gpsimd.partition_broadcast(den[:], recip[:])
                                with nc.allow_low_precision("bf16 store"):
                                    nc.vector.tensor_mul(
                                        ot_t[p][hh * D : (hh + 1) * D, qsl],
                                        otu[:],
                                        den[:],
                                    )
                            return fn

                        return [phase1(0), phase1(1), recips,
                                phase2(0), phase2(1)]

                    # flat pipeline over (chunk, head-pair, ts-tile): score
                    # matmuls for the next step fill the PE while the previous
                    # step's exp and the AV accumulation complete.  AV runs on
                    # ts-tile PAIRS via fp8 DoubleRow: pt tiles hold (slab,
                    # hh, tq) and one DR matmul per pair contracts 256 tokens.
                    tiles = [(cq, p, t)
                             for cq in range(NCHUNK)
                             for p in range(2)
                             for t in range(4 * cq + 4)]
                    ots_cur = None
                    pt_cur = None
                    norm_q = []
                    sts = {tiles[0]: emit_st(*tiles[0])}
                    for i, (cq, p, t) in enumerate(tiles):
                        nts = 4 * cq + 4
                        if t == 0:
                            ots_cur = [
                                ps_ot.tile([D + 1, CH], F32, tag="ot",
                                           name=f"ot{cq}_{p}_{hh}")
                                for hh in range(2)
                            ]
                        if i + 1 < len(tiles):
                            sts[tiles[i + 1]] = emit_st(*tiles[i + 1])
                        if norm_q:
                            norm_q.pop(0)()
                        st, js = sts.pop((cq, p, t))
                        if cq == 0:
                            # early queries: bf16 probs, bf16 V, per-tile AV
                            ptb = pt_pool.tile([P, 2, CH], BF16, tag="ptb",
                                               name=f"ptb{p}_{t}")
                            with nc.allow_low_precision("bf16 probs"):
                                nc.scalar.activation(ptb[:, :, js:],
                                                     st[:, :, js:], EXP,
                                                     scale=EXP_SCALE)
                                if not no_mask:
                                    nc.vector.tensor_mul(
                                        ptb[:, :, js : js + P],
                                        ptb[:, :, js : js + P],
                                        mask01b[:],
                                    )
                            for hh in range(2):
                                h = 2 * p + hh
                                nc.tensor.matmul(
                                    ots_cur[hh][0 : D + 1, js:],
                                    lhsT=vaugb_t[t][:, h, :],
                                    rhs=ptb[:, hh, js:],
                                    start=(t == 0),
                                    stop=(t == nts - 1),
                                )
                        else:
                            if t % 2 == 0:
                                pt_cur = pt_pool.tile([P, 2, 2, CH], F8,
                                                      tag="pt",
                                                      name=f"pt{cq}_{p}_{t}")
                                if t >= 4 * cq:
                                    # slab 1's columns [js:js+P] lie in tile
                                    # t+1's masked region: zero them so the
                                    # DR matmul (which streams both slabs
                                    # from js) adds nothing there
                                    nc.gpsimd.memset(
                                        pt_cur[:, 1, :, js : js + P], 0.0)
                            with nc.allow_low_precision("fp8 probs"):
                                nc.scalar.activation(pt_cur[:, t % 2, :, js:],
                                                     st[:, :, js:], EXP,
                                                     scale=EXP_SCALE)
                                if t >= 4 * cq and not no_mask:
                                    nc.vector.tensor_mul(
                                        pt_cur[:, t % 2, :, js : js + P],
                                        pt_cur[:, t % 2, :, js : js + P],
                                        mask01[:],
                                    )
                            if t % 2 == 1:
                                # AV for pair (t-1, t), streamed from js0
                                k = t // 2
                                js0 = max(0, (t - 1 - 4 * cq) * P)
                                for hh in range(2):
                                    h = 2 * p + hh
                                    nc.tensor.matmul(
                                        ots_cur[hh][0 : D + 1, js0:],
                                        lhsT=vaug_t[k][:, :, h, 0 : D + 1],
                                        rhs=pt_cur[:, :, hh, js0:],
                                        start=(t == 1),
                                        stop=(t == nts - 1),
                                        perf_mode=DR,
                                    )
                        if pending_out and t >= 5:
                            emit_outproj_sub(pending_out.pop(0))
                        if t == nts - 1:
                            norm_q.extend(norm_steps(
                                cq, p, ots_cur,
                                on_scalar=(i == len(tiles) - 1)))
                            if p == 1 and stages >= 3:
                                pending_out.extend(
                                    (tt, nn)
                                    for tt in range(4 * cq, 4 * cq + 4)
                                    for nn in range(2))
                    # Final drain: spread remaining out-proj subtiles while
                    # the last normalize chain completes.
                    subtiles = list(pending_out)
                    pending_out.clear()
                    head, tail2 = norm_q[:-2], norm_q[-2:]
                    for fn in head:
                        fn()

                    def alloc_yp(idx):
                        kk = idx % 6
                        if kk == 0:
                            return ps_y.tile([P, CH], F32, tag="y",
                                             name=f"yf{idx}")[:]
                        if kk in (1, 2):
                            raw = ps_st.tile([P, 2, CH], F32, tag="st",
                                             name=f"yf{idx}")
                            return raw[:, 0, :]
                        return ps_ot.tile([P, CH], F32, tag="ot",
                                          name=f"yf{idx}")[:]

                    yps = {}
                    for idx in range(min(6, len(subtiles))):
                        tt, nn = subtiles[idx]
                        yp = alloc_yp(idx)
                        nc.tensor.matmul(
                            yp,
                            lhsT=ot_t[0][:, tt * P : (tt + 1) * P],
                            rhs=wot_t[0][:, nn * CH : (nn + 1) * CH],
                            start=True,
                            stop=False,
                        )
                        yps[idx] = yp
                    for fn in tail2:
                        fn()
                    for idx, (tt, nn) in enumerate(subtiles):
                        tsl = slice(tt * P, (tt + 1) * P)
                        nsl = slice(nn * CH, (nn + 1) * CH)
                        if idx in yps:
                            yp = yps[idx]
                        else:
                            yp = alloc_yp(idx)
                            nc.tensor.matmul(
                                yp,
                                lhsT=ot_t[0][:, tsl],
                                rhs=wot_t[0][:, nsl],
                                start=True,
                                stop=False,
                            )
                        nc.tensor.matmul(
                            yp,
                            lhsT=ot_t[1][:, tsl],
                            rhs=wot_t[1][:, nsl],
                            start=False,
                            stop=True,
                        )
                        ysb = y_pool.tile([P, CH], BF16, tag="ysb",
                                          name=f"ysbf{idx}")
                        with nc.allow_low_precision("bf16 Y"):
                            nc.scalar.copy(ysb[:], yp)
                        nc.sync.dma_start(y[tsl, nsl], ysb[:])

            def emit_dbg_outputs():
                if stages == 1:
                    for i, src_t in enumerate((qt_t[0], qt_t[1], kt_t[0],
                                               kt_t[1])):
                        nc.sync.dma_start(y[i * P : (i + 1) * P, 0:C],
                                          src_t[:, 0:C].bitcast(F32))
                elif stages == 2:
                    for i, src_t in enumerate((ot_t[0], ot_t[1])):
                        nc.sync.dma_start(y[i * P : (i + 1) * P, :],
                                          src_t[:, 0:C].bitcast(F32))

            emit_stage1()
            if stages >= 2:
                emit_stage23()
            emit_dbg_outputs()

    nc.finalize()
    return nc


_NC_CACHE = None


def get_nc():
    global _NC_CACHE
    if _NC_CACHE is None:
        _NC_CACHE = build_nc()
    return _NC_CACHE


def make_in_maps(x, Wq, Wk, Wv, Wo):
    import ml_dtypes

    bf16 = ml_dtypes.bfloat16
    f8 = ml_dtypes.float8_e4m3
    scale = 1.0 / np.sqrt(np.float32(C))
    in_maps = []
    for core in range(8):
        b, hg = core // 4, core % 4
        hsl = slice(hg * NHPC, (hg + 1) * NHPC)
        # x^T in c-tile-pair layout [NCP, 128, 2, T]
        xT = np.ascontiguousarray(x[b].T)  # [C, T]
        x8 = np.ascontiguousarray(
            xT.reshape(NCP, 2, P, T).transpose(0, 2, 1, 3)).astype(f8)
        wq = (Wq[hsl] * (scale * S_Q)).transpose(1, 0, 2).reshape(C, DH)
        wk = (Wk[hsl] * S_K).transpose(1, 0, 2).reshape(C, DH)
        wv = (Wv[hsl] * S_V).transpose(1, 0, 2).reshape(C, DH)
        wqkv = np.concatenate([wq, wk, wv], axis=1, dtype=np.float32)  # [C, 768]
        w8 = np.ascontiguousarray(
            wqkv.reshape(NCP, 2, P, 3 * DH).transpose(0, 2, 1, 3)).astype(f8)
        wot = np.ascontiguousarray(
            Wo[:, hg * DH : (hg + 1) * DH].T / S_V).astype(bf16)
        xtb = np.ascontiguousarray(xT[:, 0:CH]).astype(bf16)
        wvb = np.ascontiguousarray(
            (Wv[hsl] * S_V).transpose(1, 0, 2).reshape(C, DH)).astype(bf16)
        in_maps.append({
            "X8": x8,
            "W8": w8,
            "WoT": wot,
            "XTB": xtb,
            "WVB": wvb,
        })
    return in_maps


def gather(results, bo):
    out = np.zeros((B, T, C), dtype=np.float32)
    for core in range(8):
        out[core // 4] += results[core]["Y"].astype(np.float32)
    out += bo.astype(np.float32)
    return out


def kernel(x, Wq, Wk, Wv, Wo, bo, **run_kwargs):
    x = np.asarray(x, dtype=np.float32)
    Wq = np.asarray(Wq, dtype=np.float32)
    Wk = np.asarray(Wk, dtype=np.float32)
    Wv = np.asarray(Wv, dtype=np.float32)
    Wo = np.asarray(Wo, dtype=np.float32)
    bo = np.asarray(bo, dtype=np.float32)
    nc = get_nc()
    in_maps = make_in_maps(x, Wq, Wk, Wv, Wo)
    res = run_bass_kernel_spmd(nc, in_maps, core_ids=list(range(8)), **run_kwargs)
    out = gather(res.results, bo)
    if run_kwargs:
        return out, res
    return out


# revision 32
# speedup vs baseline: 1.0590x; 1.0590x over previous
"""Trainium2 Bass kernel for 16-head causal MHA (B=2, T=2048, C=1024, H=16, D=64).

Sharding: 8 cores = 2 batch groups x 4 head groups (4 heads each).

v2: fp8e4 DoubleRow matmuls for the projections (c-tile pairs, contraction
256/instr at 0.5 cyc/row) and for the AV accumulation (ts-tile pairs).
Scores and out-projection stay bf16 in plain 128x128 mode, so the PE never
switches tiling modes.

Per-core pipeline:
  Q^T,K^T = fp8-DR projections kept transposed [dims, tokens], bf16 out
            (Wq pre-scaled by softmax-scale*1024, Wk by 32 on host)
  V       = fp8-DR projection, natural [tokens, dims], stored fp8 (Wv*32)
            in ts-tile-PAIR layout vaug[pair][:, slab, head, 0:64] plus a
            ones column per head (denominator trick)
  S^T     = bf16 K Q^T per (ts-tile, tq-chunk), causal-masked on the
            diagonal block, exp'd with scale 2^-15 (undoes S_q*S_k) into
            fp8 pt pairs
  O^T_aug = fp8-DR V_aug^T P^T over ts-tile pairs; row 64 is the softmax
            denominator; normalized via partition_broadcast + DVE
  Y_part  = O^T.T @ (Wo_slice/32)^T bf16, interleaved per chunk
Host sums the 4 head-group partials per batch and adds bo.
"""

import sys

sys.path.insert(0, "/opt/trn_rl_repo")

import numpy as np

import concourse.bass as bass
from concourse import bacc
import concourse.mybir as mybir
from concourse.tile import TileContext
from concourse.bass_utils import run_bass_kernel_spmd

F32 = mybir.dt.float32
BF16 = mybir.dt.bfloat16
F8 = mybir.dt.float8e4
DR = mybir.MatmulPerfMode.DoubleRow
EXP = mybir.ActivationFunctionType.Exp

B, T, C, H, D = 2, 2048, 1024, 16, 64
NHPC = 4          # heads per core
DH = NHPC * D     # 256 head dims per core
P = 128           # partitions
CH = 512          # token chunk (matmul moving dim)
NCHUNK = T // CH  # 4
NTT = T // P      # 16 token tiles
NCP = C // (2 * P)  # 4 contraction PAIRS over C (DoubleRow slabs)
# host-side scale factors keeping fp8 operands in e4m3 normal range
S_Q = 1024.0      # on Wq (includes softmax 1/32): exp scale undoes S_Q*S_K
S_K = 32.0
S_V = 32.0        # on Wv: undone by Wo/32 on host
EXP_SCALE = 1.0 / (S_Q * S_K)


def build_nc(stages=3, no_mask=False, no_norm=False):
    nc = bacc.Bacc()
    # x^T in c-tile-pair layout: [cpair, c-within(128 part), slab, tokens]
    x8_d = nc.declare_dram_parameter("X8", [NCP, P, 2, T], F8, isOutput=False)
    # Wqkv fp8 in c-tile-pair layout: cols 0:256 Q, 256:512 K, 512:768 V
    w8_d = nc.declare_dram_parameter("W8", [NCP, P, 2, 3 * DH], F8,
                                     isOutput=False)
    wot_d = nc.declare_dram_parameter("WoT", [DH, C], BF16, isOutput=False)
    # bf16 x^T (first 512 tokens) and Wv for the accurate early-V path
    xtb_d = nc.declare_dram_parameter("XTB", [C, CH], BF16, isOutput=False)
    wvb_d = nc.declare_dram_parameter("WVB", [C, DH], BF16, isOutput=False)
    y_d = nc.declare_dram_parameter("Y", [T, C], BF16, isOutput=True)

    wot = wot_d[:, :]
    y = y_d[:, :]

    with TileContext(nc) as tc:
        with (
            tc.tile_pool(name="const", bufs=1) as const,
            tc.tile_pool(name="persist", bufs=1) as persist,
        ):
            # ---- constants ----
            ones_f32 = const.tile([P, 1], F32)
            nc.gpsimd.memset(ones_f32[:], 1.0)
            # warm the GPSIMD library that partition_broadcast lives in
            wsrc = const.tile([1, 8], F32, name="warmsrc")
            wdst = const.tile([D, 8], F32, name="warmdst")
            nc.gpsimd.memset(wsrc[:], 1.0)
            nc.gpsimd.partition_broadcast(wdst[:], wsrc[:])
            nc.vector.tensor_copy(wsrc[:], wdst[0:1, :])
            # triangular mask for the diagonal 128x128 block (both heads):
            # mask[r, (hh, j)] = 1 if r <= j else 0, applied to fp8 probs
            mask01 = const.tile([P, 2, P], F8, name="mask01")
            mask01b = const.tile([P, 2, P], BF16, name="mask01b")
            for mt in (mask01, mask01b):
                nc.gpsimd.memset(mt[:], 1.0)
                nc.gpsimd.affine_select(
                    out=mt[:],
                    in_=mt[:],
                    compare_op=mybir.AluOpType.is_ge,
                    fill=0.0,
                    base=0,
                    pattern=[[0, 2], [1, P]],
                    channel_multiplier=-1,
                )

            # ---- persistent tensors ----
            w8_t = [persist.tile([P, 2, 3 * DH], F8, name=f"w8_{c}")
                    for c in range(NCP)]
            wot_t = [persist.tile([P, C], BF16, name=f"wot{k}") for k in range(2)]
            # Q^T/K^T [dims, tokens] bf16; pair p holds heads (2p, 2p+1)
            qt_t = [persist.tile([P, T], BF16, name=f"qt{p}") for p in range(2)]
            kt_t = [persist.tile([P, T], BF16, name=f"kt{p}") for p in range(2)]
            # V fp8 in ts-tile-pair layout + ones column per head:
            # vaug[k][:, slab, h, 0:64] = V' rows for ts-tile 2k+slab
            vaug_t = [persist.tile([P, 2, NHPC, 2 * D], F8, name=f"vaug{k}")
                      for k in range(NTT // 2)]
            # bf16 V for ts-tiles 0-3 (queries < 512 need accurate V)
            vaugb_t = [persist.tile([P, NHPC, D + 1], BF16, name=f"vaugb{t}")
                       for t in range(4)]
            # ones column at slot 64: the denominator lands on PSUM
            # partition 64 (32-aligned), where reciprocal_approx_fast reads
            # it directly -- no staging copy
            with nc.allow_low_precision("fp8 attention pipeline"):
                for k in range(NTT // 2):
                    nc.gpsimd.memset(vaug_t[k][:, :, :, D : D + 1], 1.0)
                for t in range(4):
                    nc.gpsimd.memset(vaugb_t[t][:, :, D : D + 1], 1.0)
            # normalized O^T [dims, tokens] bf16
            ot_t = [persist.tile([P, T], BF16, name=f"ot{p}") for p in range(2)]

            def emit_stage1():
                with (
                    tc.tile_pool(name="xt", bufs=12) as xt_pool,
                    tc.tile_pool(name="xtb", bufs=8) as xtb_pool,
                    tc.tile_pool(name="psproj", bufs=4, space="PSUM") as ps_proj,
                ):
                    xtb_ts = []
                    wvb_ts = []
                    for n in range(NCHUNK):
                        csl = slice(n * CH, (n + 1) * CH)
                        xts = []
                        for c in range(NCP):
                            if n == 0:
                                nc.sync.dma_start(
                                    w8_t[c][:], w8_d[c, :, :, :])
                            xtile = xt_pool.tile([P, 2, CH], F8, tag="xt",
                                                 name=f"xt{n}_{c}")
                            nc.scalar.dma_start(xtile[:],
                                                x8_d[c, :, :, csl])
                            xts.append(xtile)
                        if n == 0:
                            for k in range(2):
                                nc.sync.dma_start(wot_t[k][:],
                                                  wot[k * P : (k + 1) * P, :])
                            for c8 in range(8):
                                xb = xtb_pool.tile([P, CH], BF16, tag="xb",
                                                   name=f"xtb{c8}")
                                nc.sync.dma_start(
                                    xb[:], xtb_d[c8 * P : (c8 + 1) * P, :])
                                xtb_ts.append(xb)
                                wb = xtb_pool.tile([P, DH], BF16, tag="wb",
                                                   name=f"wvb{c8}")
                                nc.sync.dma_start(
                                    wb[:], wvb_d[c8 * P : (c8 + 1) * P, :])
                                wvb_ts.append(wb)
                        # Q^T/K^T: W stationary, x^T moving -> [dims, tokens]
                        for m in range(4):
                            ps = ps_proj.tile([P, CH], F32, tag="ps",
                                              name=f"ps{n}_{m}")
                            for c in range(NCP):
                                nc.tensor.matmul(
                                    ps[:],
                                    lhsT=w8_t[c][:, :, m * P : (m + 1) * P],
                                    rhs=xts[c][:],
                                    start=(c == 0),
                                    stop=(c == NCP - 1),
                                    perf_mode=DR,
                                )
                            with nc.allow_low_precision("bf16 Q/K"):
                                if m < 2:
                                    nc.vector.tensor_copy(qt_t[m][:, csl], ps[:])
                                else:
                                    nc.scalar.copy(kt_t[m - 2][:, csl], ps[:])
                        # V natural: x^T tile stationary, Wv moving.
                        # Chunk 0 (ts-tiles 0-3) uses bf16 inputs: those V
                        # rows weight early queries heavily and must be
                        # accurate.  It feeds both the bf16 vaugb tiles and
                        # the fp8 pair tiles.
                        for j in range(4):
                            t_glob = 4 * n + j
                            vp = ps_proj.tile([P, DH], F32, tag="vp",
                                              name=f"vp{n}_{j}")
                            if n == 0:
                                for c8 in range(8):
                                    nc.tensor.matmul(
                                        vp[:],
                                        lhsT=xtb_ts[c8][:, j * P : (j + 1) * P],
                                        rhs=wvb_ts[c8][:],
                                        start=(c8 == 0),
                                        stop=(c8 == 7),
                                    )
                            else:
                                for c in range(NCP):
                                    nc.tensor.matmul(
                                        vp[:],
                                        lhsT=xts[c][:, :, j * P : (j + 1) * P],
                                        rhs=w8_t[c][:, :, 2 * DH : 3 * DH],
                                        start=(c == 0),
                                        stop=(c == NCP - 1),
                                        perf_mode=DR,
                                    )
                            # strided copies: heads land at 65/128-col slots
                            va = vaug_t[t_glob // 2]
                            with nc.allow_low_precision("fp8 V"):
                                nc.vector.tensor_copy(
                                    va[:, t_glob % 2, :, 0:D],
                                    vp[:].rearrange("p (h d) -> p h d",
                                                    h=NHPC))
                                if n == 0:
                                    nc.scalar.copy(
                                        vaugb_t[j][:, :, 0:D],
                                        vp[:].rearrange("p (h d) -> p h d",
                                                        h=NHPC))

            def emit_stage23():
                with (
                    tc.tile_pool(name="pt", bufs=4) as pt_pool,
                    tc.tile_pool(name="small", bufs=4) as small_pool,
                    tc.tile_pool(name="ysb", bufs=3) as y_pool,
                    tc.tile_pool(name="psst", bufs=2, space="PSUM") as ps_st,
                    tc.tile_pool(name="psot", bufs=3, space="PSUM") as ps_ot,
                    tc.tile_pool(name="psy", bufs=1, space="PSUM") as ps_y,
                ):
                    pending_out = []

                    def emit_outproj_sub(sub, on_scalar=False):
                        tt, nn = sub
                        tsl = slice(tt * P, (tt + 1) * P)
                        nsl = slice(nn * CH, (nn + 1) * CH)
                        yp = ps_y.tile([P, CH], F32, tag="y",
                                       name=f"y{tt}_{nn}")
                        for k in range(2):
                            nc.tensor.matmul(
                                yp[:],
                                lhsT=ot_t[k][:, tsl],
                                rhs=wot_t[k][:, nsl],
                                start=(k == 0),
                                stop=(k == 1),
                            )
                        ysb = y_pool.tile([P, CH], BF16, tag="ysb",
                                          name=f"ysb{tt}_{nn}")
                        with nc.allow_low_precision("bf16 Y"):
                            if on_scalar:
                                nc.scalar.copy(ysb[:], yp[:])
                            else:
                                nc.vector.tensor_copy(ysb[:], yp[:])
                        nc.sync.dma_start(y[tsl, nsl], ysb[:])

                    def emit_st(cq, p, t):
                        st = ps_st.tile([P, 2, CH], F32, tag="st",
                                        name=f"st{cq}_{p}_{t}")
                        tsl = slice(t * P, (t + 1) * P)
                        js = max(0, (t - 4 * cq) * P)
                        qs = slice(cq * CH + js, (cq + 1) * CH)
                        for hh in range(2):
                            nc.tensor.matmul(
                                st[:, hh, js:],
                                lhsT=kt_t[p][hh * D : (hh + 1) * D, tsl],
                                rhs=qt_t[p][hh * D : (hh + 1) * D, qs],
                                start=True,
                                stop=True,
                            )
                        return st, js

                    def norm_steps(cq, p, ots, on_scalar=False):
                        """Normalize chain for (cq, p) as separate steps so
                        they interleave with the pipeline on the DVE queue."""
                        qsl = slice(cq * CH, (cq + 1) * CH)
                        if no_norm:
                            def all_copies():
                                with nc.allow_low_precision("timing variant"):
                                    for hh in range(2):
                                        nc.vector.tensor_copy(
                                            ot_t[p][hh * D : (hh + 1) * D, qsl],
                                            ots[hh][0:D, :])
                            return [all_copies]
                        cp = nc.scalar.copy if on_scalar else nc.vector.tensor_copy
                        state = {}

                        def phase1(hh):
                            def fn():
                                ot = ots[hh]
                                # reciprocal reads the denominator row
                                # directly from PSUM partition 0; the otu
                                # bounce frees the PSUM slot quickly
                                dstg = small_pool.tile([1, CH], F32,
                                                       tag="ds",
                                                       name=f"ds{cq}_{p}_{hh}")
                                cp(dstg[:], ot[D : D + 1, :])
                                recip = small_pool.tile([1, CH], F32, tag="rc",
                                                        name=f"rc{cq}_{p}_{hh}")
                                nc.vector.reciprocal_approx_fast(
                                    recip[:], dstg[:])
                                otu = small_pool.tile([D, CH], F32,
                                                      tag="otu",
                                                      name=f"otu{cq}_{p}_{hh}")
                                cp(otu[:], ot[0:D, :])
                                state[hh] = (recip, otu)
                            return fn

                        def recips():
                            pass

                        def phase2(hh):
                            def fn():
                                recip, otu = state[hh]
                                den = small_pool.tile([D, CH], F32, tag="den",
                                                      name=f"dn{cq}_{p}_{hh}")
                                nc.gpsimd.partition_broadcast(den[:], recip[:])
                                with nc.allow_low_precision("bf16 store"):
                                    nc.vector.tensor_mul(
                                        ot_t[p][hh * D : (hh + 1) * D, qsl],
                                        otu[:],
                                        den[:],
                                    )
                            return fn

                        return [phase1(0), phase1(1), recips,
                                phase2(0), phase2(1)]

                    # flat pipeline over (chunk, head-pair, ts-tile): score
                    # matmuls for the next step fill the PE while the previous
                    # step's exp and the AV accumulation complete.  AV runs on
                    # ts-tile PAIRS via fp8 DoubleRow: pt tiles hold (slab,
                    # hh, tq) and one DR matmul per pair contracts 256 tokens.
                    tiles = [(cq, p, t)
                             for cq in range(NCHUNK)
                             for p in range(2)
                             for t in range(4 * cq + 4)]
                    ots_cur = None
                    pt_cur = None
                    norm_q = []
                    sts = {tiles[0]: emit_st(*tiles[0])}
                    for i, (cq, p, t) in enumerate(tiles):
                        nts = 4 * cq + 4
                        if t == 0:
                            ots_cur = [
                                ps_ot.tile([D + 1, CH], F32, tag="ot",
                                           name=f"ot{cq}_{p}_{hh}")
                                for hh in range(2)
                            ]
                        if i + 1 < len(tiles):
                            sts[tiles[i + 1]] = emit_st(*tiles[i + 1])
                        if norm_q:
                            norm_q.pop(0)()
                        st, js = sts.pop((cq, p, t))
                        if cq == 0:
                            # early queries: bf16 probs, bf16 V, per-tile AV
                            ptb = pt_pool.tile([P, 2, CH], BF16, tag="ptb",
                                               name=f"ptb{p}_{t}")
                            with nc.allow_low_precision("bf16 probs"):
                                nc.scalar.activation(ptb[:, :, js:],
                                                     st[:, :, js:], EXP,
                                                     scale=EXP_SCALE)
                                if not no_mask:
                                    nc.vector.tensor_mul(
                                        ptb[:, :, js : js + P],
                                        ptb[:, :, js : js + P],
                                        mask01b[:],
                                    )
                            for hh in range(2):
                                h = 2 * p + hh
                                nc.tensor.matmul(
                                    ots_cur[hh][0 : D + 1, js:],
                                    lhsT=vaugb_t[t][:, h, :],
                                    rhs=ptb[:, hh, js:],
                                    start=(t == 0),
                                    stop=(t == nts - 1),
                                )
                        else:
                            if t % 2 == 0:
                                pt_cur = pt_pool.tile([P, 2, 2, CH], F8,
                                                      tag="pt",
                                                      name=f"pt{cq}_{p}_{t}")
                                if t >= 4 * cq:
                                    # slab 1's columns [js:js+P] lie in tile
                                    # t+1's masked region: zero them so the
                                    # DR matmul (which streams both slabs
                                    # from js) adds nothing there
                                    nc.gpsimd.memset(
                                        pt_cur[:, 1, :, js : js + P], 0.0)
                            with nc.allow_low_precision("fp8 probs"):
                                nc.scalar.activation(pt_cur[:, t % 2, :, js:],
                                                     st[:, :, js:], EXP,
                                                     scale=EXP_SCALE)
                                if t >= 4 * cq and not no_mask:
                                    nc.vector.tensor_mul(
                                        pt_cur[:, t % 2, :, js : js + P],
                                        pt_cur[:, t % 2, :, js : js + P],
                                        mask01[:],
                                    )
                            if t % 2 == 1:
                                # AV for pair (t-1, t), streamed from js0
                                k = t // 2
                                js0 = max(0, (t - 1 - 4 * cq) * P)
                                for hh in range(2):
                                    h = 2 * p + hh
                                    nc.tensor.matmul(
                                        ots_cur[hh][0 : D + 1, js0:],
                                        lhsT=vaug_t[k][:, :, h, 0 : D + 1],
                                        rhs=pt_cur[:, :, hh, js0:],
                                        start=(t == 1),
                                        stop=(t == nts - 1),
                                        perf_mode=DR,
                                    )
                        if pending_out and t >= 5:
                            emit_outproj_sub(pending_out.pop(0))
                        if t == nts - 1:
                            norm_q.extend(norm_steps(
                                cq, p, ots_cur,
                                on_scalar=(i == len(tiles) - 1)))
                            if p == 1 and stages >= 3:
                                pending_out.extend(
                                    (tt, nn)
                                    for tt in range(4 * cq, 4 * cq + 4)
                                    for nn in range(2))
                    # Final drain: spread remaining out-proj subtiles while
                    # the last normalize chain completes.
                    subtiles = list(pending_out)
                    pending_out.clear()
                    head, tail2 = norm_q[:-2], norm_q[-2:]
                    for fn in head:
                        fn()

                    def alloc_yp(idx):
                        kk = idx % 6
                        if kk == 0:
                            return ps_y.tile([P, CH], F32, tag="y",
                                             name=f"yf{idx}")[:]
                        if kk in (1, 2):
                            raw = ps_st.tile([P, 2, CH], F32, tag="st",
                                             name=f"yf{idx}")
                            return raw[:, 0, :]
                        return ps_ot.tile([P, CH], F32, tag="ot",
                                          name=f"yf{idx}")[:]

                    yps = {}
                    for idx in range(min(6, len(subtiles))):
                        tt, nn = subtiles[idx]
                        yp = alloc_yp(idx)
                        nc.tensor.matmul(
                            yp,
                            lhsT=ot_t[0][:, tt * P : (tt + 1) * P],
                            rhs=wot_t[0][:, nn * CH : (nn + 1) * CH],
                            start=True,
                            stop=False,
                        )
                        yps[idx] = yp
                    for fn in tail2:
                        fn()
                    for idx, (tt, nn) in enumerate(subtiles):
                        tsl = slice(tt * P, (tt + 1) * P)
                        nsl = slice(nn * CH, (nn + 1) * CH)
                        if idx in yps:
                            yp = yps[idx]
                        else:
                            yp = alloc_yp(idx)
                            nc.tensor.matmul(
                                yp,
                                lhsT=ot_t[0][:, tsl],
                                rhs=wot_t[0][:, nsl],
                                start=True,
                                stop=False,
                            )
                        nc.tensor.matmul(
                            yp,
                            lhsT=ot_t[1][:, tsl],
                            rhs=wot_t[1][:, nsl],
                            start=False,
                            stop=True,
                        )
                        ysb = y_pool.tile([P, CH], BF16, tag="ysb",
                                          name=f"ysbf{idx}")
                        with nc.allow_low_precision("bf16 Y"):
                            nc.scalar.copy(ysb[:], yp)
                        nc.sync.dma_start(y[tsl, nsl], ysb[:])

            def emit_dbg_outputs():
                if stages == 1:
                    for i, src_t in enumerate((qt_t[0], qt_t[1], kt_t[0],
                                               kt_t[1])):
                        nc.sync.dma_start(y[i * P : (i + 1) * P, 0:C],
                                          src_t[:, 0:C].bitcast(F32))
                elif stages == 2:
                    for i, src_t in enumerate((ot_t[0], ot_t[1])):
                        nc.sync.dma_start(y[i * P : (i + 1) * P, :],
                                          src_t[:, 0:C].bitcast(F32))

            emit_stage1()
            if stages >= 2:
                emit_stage23()
            emit_dbg_outputs()

    nc.finalize()
    return nc


_NC_CACHE = None


def get_nc():
    global _NC_CACHE
    if _NC_CACHE is None:
        _NC_CACHE = build_nc()
    return _NC_CACHE


def make_in_maps(x, Wq, Wk, Wv, Wo):
    import ml_dtypes

    bf16 = ml_dtypes.bfloat16
    f8 = ml_dtypes.float8_e4m3
    scale = 1.0 / np.sqrt(np.float32(C))
    in_maps = []
    for core in range(8):
        b, hg = core // 4, core % 4
        hsl = slice(hg * NHPC, (hg + 1) * NHPC)
        # x^T in c-tile-pair layout [NCP, 128, 2, T]
        xT = np.ascontiguousarray(x[b].T)  # [C, T]
        x8 = np.ascontiguousarray(
            xT.reshape(NCP, 2, P, T).transpose(0, 2, 1, 3)).astype(f8)
        wq = (Wq[hsl] * (scale * S_Q)).transpose(1, 0, 2).reshape(C, DH)
        wk = (Wk[hsl] * S_K).transpose(1, 0, 2).reshape(C, DH)
        wv = (Wv[hsl] * S_V).transpose(1, 0, 2).reshape(C, DH)
        wqkv = np.concatenate([wq, wk, wv], axis=1, dtype=np.float32)  # [C, 768]
        w8 = np.ascontiguousarray(
            wqkv.reshape(NCP, 2, P, 3 * DH).transpose(0, 2, 1, 3)).astype(f8)
        wot = np.ascontiguousarray(
            Wo[:, hg * DH : (hg + 1) * DH].T / S_V).astype(bf16)
        xtb = np.ascontiguousarray(xT[:, 0:CH]).astype(bf16)
        wvb = np.ascontiguousarray(
            (Wv[hsl] * S_V).transpose(1, 0, 2).reshape(C, DH)).astype(bf16)
        in_maps.append({
            "X8": x8,
            "W8": w8,
            "WoT": wot,
            "XTB": xtb,
            "WVB": wvb,
        })
    return in_maps


def gather(results, bo):
    out = np.zeros((B, T, C), dtype=np.float32)
    for core in range(8):
        out[core // 4] += results[core]["Y"].astype(np.float32)
    out += bo.astype(np.float32)
    return out


def kernel(x, Wq, Wk, Wv, Wo, bo, **run_kwargs):
    x = np.asarray(x, dtype=np.float32)
    Wq = np.asarray(Wq, dtype=np.float32)
    Wk = np.asarray(Wk, dtype=np.float32)
    Wv = np.asarray(Wv, dtype=np.float32)
    Wo = np.asarray(Wo, dtype=np.float32)
    bo = np.asarray(bo, dtype=np.float32)
    nc = get_nc()
    in_maps = make_in_maps(x, Wq, Wk, Wv, Wo)
    res = run_bass_kernel_spmd(nc, in_maps, core_ids=list(range(8)), **run_kwargs)
    out = gather(res.results, bo)
    if run_kwargs:
        return out, res
    return out
